# revision 14
# baseline (speedup 1.0000x reference)
"""Causal self-attention (RoPE-3D + QK-RMSNorm) on 8 TRN2 NeuronCores.

Tensor-parallel over heads: 2 heads per core. Host shards W_qkv rows /
W_out columns, replicates x (pre-transposed, bf16), precomputes fused
RoPE cos/sin tables, and sums the 8 per-core partial projection outputs.

Per-core device kernel (Bass/Tile, no collectives), all-bf16 matmuls:
  Phase A (per 512-token block): QKV projection from a single bf16 x
    stream, QK-RMSNorm via ones-matmul partition reduction + Rsqrt +
    matmul partition-broadcast, RoPE via resident fused tables +
    stream_shuffle. q,k feature-major bf16 [96, tokens]; v token-major
    bf16 [tokens, 96(+1 ones col)].
  Phase B (per 512 q-token i-block): S^T = k_tile^T q_block (bf16),
    exp on ACT (no max subtraction needed: |S| <= sqrt(D)), causal mask
    via affine_select with diagonal-tile column trimming, AV + softmax
    denominator via [v|1] matmul, 1/Z on DVE reciprocal_approx_fast,
    out-projection into a [C, tokens] partial the host sums across
    cores.
"""

import math
import os
from contextlib import ExitStack

import numpy as np
import ml_dtypes

import concourse.bass as bass
import concourse.mybir as mybir
import concourse.tile as tile
from concourse import bacc
from concourse.bass_utils import run_bass_kernel_spmd

B, T, C = 2, 2048, 1536
H, D = 16, 96
NT = B * T                    # 4096 tokens
NCORES = 8
HPC = H // NCORES             # heads per core
ROPE_BASE = 10000.0

F32 = mybir.dt.float32
BF16 = mybir.dt.bfloat16

KT = C // 128                 # 12 contraction tiles over C
NBLK = NT // 512              # 8 token blocks
IB_PER_B = T // 512           # 4 q i-blocks per batch
VSTRIDE = 32 * 97             # v_sb per-head columns: 32 token-tiles x (96+1)

_CACHE = {}


# ----------------------------------------------------------------- host side

def _host_tables(coords, token_type, q_scale, k_scale):
    tt = (np.asarray(token_type).reshape(NT) > 0)
    half = 16
    inv_freq = ROPE_BASE ** (-np.arange(half, dtype=np.float64) / half)
    cf = np.empty((NT, D), np.float64)
    sf = np.empty((NT, D), np.float64)
    cflat = np.asarray(coords).reshape(NT, 3).astype(np.float64)
    for a in range(3):
        ang = cflat[:, a:a + 1] * inv_freq[None, :]
        c, s = np.cos(ang), np.sin(ang)
        cf[:, a * 32:a * 32 + 16] = c
        cf[:, a * 32 + 16:a * 32 + 32] = c
        sf[:, a * 32:a * 32 + 16] = -s
        sf[:, a * 32 + 16:a * 32 + 32] = s
    cf[~tt] = 1.0
    sf[~tt] = 0.0
    pi = (np.arange(D) // 32) * 32 + (np.arange(D) + 16) % 32
    c0 = 1.0 / math.sqrt(D)
    q_scale = np.asarray(q_scale, np.float64)
    k_scale = np.asarray(k_scale, np.float64)
    bf = ml_dtypes.bfloat16
    cosq = np.ascontiguousarray((cf * (q_scale[None, :] * c0)).T).astype(bf)
    sinq = np.ascontiguousarray((sf * (q_scale[pi][None, :] * c0)).T).astype(bf)
    cosk = np.ascontiguousarray((cf * k_scale[None, :]).T).astype(bf)
    sink = np.ascontiguousarray((sf * k_scale[pi][None, :]).T).astype(bf)
    return cosq, sinq, cosk, sink


def _make_in_maps(x, coords, token_type, W_qkv, W_out, q_scale, k_scale):
    x = np.asarray(x, np.float32)
    W_qkv = np.asarray(W_qkv, np.float32)
    W_out = np.asarray(W_out, np.float32)
    xbT = np.ascontiguousarray(x.reshape(NT, C).T).astype(ml_dtypes.bfloat16)
    cosq, sinq, cosk, sink = _host_tables(coords, token_type, q_scale, k_scale)
    in_maps = []
    for ci in range(NCORES):
        h0 = HPC * ci
        rows = np.concatenate([
            W_qkv[h0 * D:(h0 + HPC) * D],
            W_qkv[C + h0 * D:C + (h0 + HPC) * D],
        ], axis=0)                                        # [384, C] q,k rows
        wqkvT = np.ascontiguousarray(rows.T).astype(ml_dtypes.bfloat16)
        wvT = np.ascontiguousarray(
            W_qkv[2 * C + h0 * D:2 * C + (h0 + HPC) * D].T
        ).astype(ml_dtypes.bfloat16)                      # [C, 192] bf16
        woT = np.ascontiguousarray(
            W_out[:, h0 * D:(h0 + HPC) * D].T
        ).astype(ml_dtypes.bfloat16)                      # [192, C] bf16
        in_maps.append({
            "xbT": xbT, "wqkvT": wqkvT, "wvT": wvT, "woT": woT,
            "cosq": cosq, "sinq": sinq, "cosk": cosk, "sink": sink,
        })
    return in_maps


# --------------------------------------------------------------- bass builder

SWAP16 = [(i + 16) % 32 for i in range(32)]

_COMBINED_ACT_SET = "natural_log_exp_and_others"


def _pin_act_tables():
    """Make the ACT table-set picker choose the combined exp+ln set for
    every Exp/Ln activation (one ACT_TABLE_LOAD for the whole kernel
    instead of two switches per block). Only set *membership* is masked;
    dict order — and therefore act_func_set_id numbering — is unchanged."""
    import concourse.bacc as bacc_mod
    import concourse.hw_specs as hw_specs
    if getattr(bacc_mod, "_act_tables_pinned", False):
        return
    orig = hw_specs.get_activation_tables
    AF = mybir.ActivationFunctionType

    def patched(arch):
        tables = orig(arch)
        if _COMBINED_ACT_SET in tables:
            for name, fns in tables.items():
                if name != _COMBINED_ACT_SET:
                    fns.discard(AF.Exp)
                    fns.discard(AF.Ln)
        return tables

    bacc_mod.get_activation_tables = patched
    bacc_mod._act_tables_pinned = True


def _build():
    _pin_act_tables()
    nc = bacc.Bacc("TRN2", target_bir_lowering=False, debug=False)
    AF = mybir.ActivationFunctionType

    xbT = nc.declare_dram_parameter("xbT", [C, NT], BF16, isOutput=False)
    wqkvT = nc.declare_dram_parameter("wqkvT", [C, 2 * HPC * D], BF16, isOutput=False)
    wvT = nc.declare_dram_parameter("wvT", [C, HPC * D], BF16, isOutput=False)
    woT = nc.declare_dram_parameter("woT", [HPC * D, C], BF16, isOutput=False)
    cosq = nc.declare_dram_parameter("cosq", [D, NT], BF16, isOutput=False)
    sinq = nc.declare_dram_parameter("sinq", [D, NT], BF16, isOutput=False)
    cosk = nc.declare_dram_parameter("cosk", [D, NT], BF16, isOutput=False)
    sink = nc.declare_dram_parameter("sink", [D, NT], BF16, isOutput=False)
    outT = nc.declare_dram_parameter("outT", [C, NT], BF16, isOutput=True)
    KDEBUG = bool(os.environ.get("KDEBUG"))
    TRIM = os.environ.get("KTRIM", "1") == "1"
    if KDEBUG:
        qTd = nc.declare_dram_parameter("qTd", [96, HPC * NT], BF16, isOutput=True)
        kTd = nc.declare_dram_parameter("kTd", [96, HPC * NT], BF16, isOutput=True)
        vd = nc.declare_dram_parameter("vd", [128, HPC * VSTRIDE], BF16, isOutput=True)
        rinvKd = nc.declare_dram_parameter("rinvKd", [128, HPC * 32], F32, isOutput=True)

    with ExitStack() as ctx:
        tc = ctx.enter_context(tile.TileContext(nc))
        resid = ctx.enter_context(tc.tile_pool(name="resid", bufs=1))
        xbp = ctx.enter_context(tc.tile_pool(name="xbp", bufs=8))
        wk = ctx.enter_context(tc.tile_pool(name="wk", bufs=4))
        wk2 = ctx.enter_context(tc.tile_pool(name="wk2", bufs=4))
        rq = ctx.enter_context(tc.tile_pool(name="rq", bufs=3))
        rk = ctx.enter_context(tc.tile_pool(name="rk", bufs=2))
        zp = ctx.enter_context(tc.tile_pool(name="zp", bufs=3))
        ep = ctx.enter_context(tc.tile_pool(name="ep", bufs=4))
        op_ = ctx.enter_context(tc.tile_pool(name="op", bufs=5))
        up_ = ctx.enter_context(tc.tile_pool(name="up", bufs=3))
        obp = ctx.enter_context(tc.tile_pool(name="obp", bufs=2))
        bp = ctx.enter_context(tc.tile_pool(name="bp", bufs=3))
        psA = ctx.enter_context(tc.tile_pool(name="psA", bufs=2, space="PSUM"))
        psS = ctx.enter_context(tc.tile_pool(name="psS", bufs=2, space="PSUM"))
        psU = ctx.enter_context(tc.tile_pool(name="psU", bufs=1, space="PSUM"))
        psW = ctx.enter_context(tc.tile_pool(name="psW", bufs=3, space="PSUM"))
        dp = ctx.enter_context(tc.tile_pool(name="dp", bufs=4, space="DRAM"))

        # ---- residents. DMA descriptor issue is serialized per engine
        # queue (~0.8us each), so spread the startup loads across engine
        # queues and order them so the first qk matmul can start as soon
        # as wq chunk 0 + x0 tile 0 land (~3us) instead of after all
        # resident loads (~19us). wq is split into 4 chunks paired with
        # the 4 x tiles of block 0; x block 0 goes on the idle vector
        # queue so its issue overlaps the wq issues on sync.
        wq_sb = resid.tile([128, KT * 384], BF16, tag="wq")
        for a in range(4):
            nc.sync.dma_start(
                out=wq_sb[:, a * 3 * 384:(a + 1) * 3 * 384].rearrange(
                    "p (a f) -> p a f", a=3),
                in_=wqkvT[a * 384:(a + 1) * 384, :].rearrange(
                    "(a p) f -> p a f", p=128),
            )
        wv_sb = resid.tile([128, KT * 192], BF16, tag="wv")
        wo_sb = resid.tile([96, HPC * C], BF16, tag="wo")

        def load_wv_wo():
            # gpsimd queue: keeps the scalar queue free for the stream-0
            # Square/rinv ACT chain (DMA descriptor issue is ~0.7us each)
            nc.gpsimd.dma_start(
                out=wv_sb[:].rearrange("p (a f) -> p a f", a=KT),
                in_=wvT[:, :].rearrange("(a p) f -> p a f", p=128),
            )
            nc.gpsimd.dma_start(
                out=wo_sb[:].rearrange("p (h f) -> p h f", h=HPC),
                in_=woT[:, :].rearrange("(h p) f -> p h f", p=96),
            )
        qT_sb = resid.tile([96, HPC * NT], BF16, tag="qT")
        kT_sb = resid.tile([96, HPC * NT], BF16, tag="kT")
        v_sb = resid.tile([128, HPC * VSTRIDE], BF16, tag="v")
        rinvK_sb = resid.tile([128, HPC * 32], F32, tag="rinvK")
        onesb_sb = resid.tile([128, 1], BF16, tag="onesb")
        nc.gpsimd.memset(onesb_sb[:], 1.0)
        eps_sb = resid.tile([1, 1], F32, tag="eps")
        nc.gpsimd.memset(eps_sb[:], 1e-6)
        # ones column of each v token-tile
        nc.gpsimd.memset(
            v_sb[:].rearrange("p (h t f) -> p h t f", h=HPC, t=32)[:, :, :, 96:97],
            1.0,
        )

        # resident RoPE tables: block-0 columns first (rope(0) needs them
        # ~5us in), the remainder deferred into streams 0-2 so the bulk
        # doesn't steal HBM bandwidth from the critical wq/x0 transfers
        tab_sb = {}
        TABS = (("cq", cosq), ("sq", sinq), ("ck", cosk), ("sk", sink))

        def load_tables():
            for nm, par in TABS:
                t = resid.tile([96, NT], BF16, tag=f"tab{nm}")
                nc.gpsimd.dma_start(out=t[:, 0:512], in_=par[0:96, 0:512])
                tab_sb[nm] = t

        def load_tables_chunk(lo, hi):
            for nm, par in TABS:
                nc.scalar.dma_start(out=tab_sb[nm][:, lo:hi],
                                    in_=par[0:96, lo:hi])

        # ---------------------------------------------------------- units

        def dma_unit(n, st, engs=None):
            """Load 4 bf16 x tiles [128, 3*512] for token block n."""
            for g3 in range(KT // 3):
                tb = xbp.tile([128, 3 * 512], BF16, tag="xb", name=f"xb{g3}")
                eng = engs[g3] if engs else nc.sync
                eng.dma_start(
                    out=tb[:].rearrange("p (a f) -> p a f", a=3),
                    in_=xbT[g3 * 384:(g3 + 1) * 384,
                            n * 512:(n + 1) * 512].rearrange(
                                "(a p) f -> p a f", p=128))
                st["xbg"].append(tb)

        def mk_qk(n, st, g):
            def u():
                pa = psA.tile([96, 512], F32, tag="pA", name=f"pa{g}")
                for kt in range(KT):
                    nc.tensor.matmul(
                        pa[:],
                        lhsT=wq_sb[:, kt * 384 + g * 96:kt * 384 + g * 96 + 96],
                        rhs=st["xbg"][kt // 3][:, (kt % 3) * 512:
                                               (kt % 3 + 1) * 512],
                        start=(kt == 0), stop=(kt == KT - 1),
                    )
                sq = wk.tile([96, 512], BF16, tag="sq", name=f"sq{g}")
                nc.scalar.activation(sq[:], pa[:], AF.Square)
                st["pas"][g] = pa
                st["sqs"][g] = sq
            return u

        def mk_rope(n, st, g):
            def u():
                qk = "q" if g < HPC else "k"
                hh = g % HPC
                pa = st["pas"][g]
                ctab = tab_sb["cq" if qk == "q" else "ck"]
                stab = tab_sb["sq" if qk == "q" else "sk"]
                csl = ctab[:, n * 512:(n + 1) * 512]
                ssl = stab[:, n * 512:(n + 1) * 512]
                m1 = wk2.tile([96, 512], BF16, tag="m1")
                nc.vector.tensor_mul(m1[:], pa[:], csl)
                qshf = wk2.tile([96, 512], F32, tag="qshf")
                nc.vector.stream_shuffle(qshf[:], pa[:], mask=SWAP16)
                qsh = wk2.tile([96, 512], BF16, tag="qsh")
                nc.vector.tensor_mul(qsh[:], qshf[:], ssl)
                dest = (qT_sb if qk == "q" else kT_sb)
                nc.vector.tensor_add(
                    dest[:, hh * NT + n * 512:hh * NT + (n + 1) * 512],
                    m1[:], qsh[:])
            return u

        def mk_v(n, st, m):
            def u():
                pv = psW.tile([128, HPC * 96], F32, tag="pW", name=f"pv{m}")
                for kt in range(KT):
                    nc.tensor.matmul(
                        pv[:],
                        lhsT=st["xbg"][kt // 3][:, (kt % 3) * 512 + m * 128:
                                                (kt % 3) * 512 + (m + 1) * 128],
                        rhs=wv_sb[:, kt * 192:(kt + 1) * 192],
                        start=(kt == 0), stop=(kt == KT - 1),
                    )
                tt = n * 4 + m
                for hh in range(HPC):
                    nc.vector.tensor_copy(
                        v_sb[:, hh * VSTRIDE + tt * 97:hh * VSTRIDE + tt * 97 + 96],
                        pv[:, hh * 96:(hh + 1) * 96])
            return u

        def mk_rinv(n, st, gs):
            """ssq reduction + rinv (= exp(-0.5*ln(mean+eps)); Ln and Exp
            share one ACT table set so no table switches) for head group
            gs, emitted mid-stream right after its rope so nothing is left
            on the stream boundary. k-side rows are transposed to column
            layout via a small DRAM round trip."""
            def u():
                for g in gs:
                    ssq = psW.tile([1, 512], F32, tag="pW", name=f"ssq{g}")
                    nc.tensor.matmul(ssq[:], lhsT=onesb_sb[0:96, 0:1],
                                     rhs=st["sqs"][g][:], start=True, stop=True)
                    lt = rk.tile([1, 512], F32, tag="lt", name=f"lt{g}")
                    nc.scalar.activation(lt[:], ssq[:], AF.Ln,
                                         scale=1.0 / D, bias=eps_sb[:])
                    if g < HPC:
                        rinv = rq.tile([1, 512], BF16, tag="row",
                                       name=f"rinv{g}")
                    else:
                        rinv = rk.tile([1, 512], F32, tag="rowk",
                                       name=f"rinv{g}")
                    nc.scalar.activation(rinv[:], lt[:], AF.Exp, scale=-0.5)
                    st["rinvs"][g] = rinv
                    if g >= HPC:
                        hh = g - HPC
                        scr = dp.tile([512], F32, tag="scr", name=f"scr{hh}")
                        nc.sync.dma_start(out=scr[:], in_=rinv[:])
                        nc.sync.dma_start(
                            out=rinvK_sb[:, hh * 32 + n * 4:
                                         hh * 32 + (n + 1) * 4],
                            in_=scr[:].rearrange("(c j) -> j c", j=128),
                        )
            return u

        def a_units(n, st):
            units = []
            for g in range(2 * HPC):
                units.append(mk_qk(n, st, g))
                units.append(mk_rope(n, st, g))
                if g == HPC - 1:
                    units.append(mk_rinv(n, st, range(HPC)))
                elif g == 2 * HPC - 1:
                    units.append(mk_rinv(n, st, range(HPC, 2 * HPC)))
            # q-norm applied in the SAME stream (right after the k-side
            # rinv unit) so attention on block n never has to wait for
            # anything emitted in stream n+1.
            units += qnorm_units(n, st)
            units += [mk_v(n, st, m) for m in range(4)]
            return units

        def qnorm_units(n, st):
            """Apply q RMS-norm to the rope output of block n in place."""
            def u():
                for g in range(HPC):
                    bq = bp.tile([96, 512], BF16, tag="bq", name=f"bq{g}")
                    nc.gpsimd.partition_broadcast(bq[:], st["rinvs"][g][:])
                    dslice = qT_sb[:, g * NT + n * 512:g * NT + (n + 1) * 512]
                    nc.vector.tensor_mul(dslice, dslice, bq[:])
            return [u]

        def attn_units(b, ib, ous_out, state=None, jlo=0, jhi=None,
                       do_tails=True):
            """Attention units for q i-block ib of batch b, j-tiles
            [jlo, jhi). Tail units compute the fully normalized ou tiles
            (appended to ous_out). `state` carries the PSUM accumulators
            when the j-range is split across emission points."""
            tok0 = b * T + ib * 512
            njt = 4 * ib + 4
            if jhi is None:
                jhi = njt
            units = []
            if state is None:
                state = {}

            def do_av(hh, jt):
                """AV accumulate for a j-tile whose es is already queued —
                emitted one unit behind the S matmul so the PE never
                head-of-line blocks on the exp."""
                ups = state[hh]
                es, c0, gt = state.pop(("es", hh, jt))
                nc.tensor.matmul(
                    ups[:, c0:512],
                    lhsT=v_sb[:, hh * VSTRIDE + gt * 97:
                              hh * VSTRIDE + gt * 97 + 97],
                    rhs=es[:, c0:512],
                    start=(jt == 0), stop=(jt == njt - 1),
                )

            def mk_j(hh, jt):
                def u():
                    if jt == 0:
                        state[hh] = psU.tile([97, 512], F32, tag="pS",
                                             name=f"ups{hh}")
                    s = jt - 4 * ib
                    c0 = 128 * s if (s > 0 and TRIM) else 0
                    sps = psS.tile([128, 512], F32, tag="pA2")
                    jtok = b * T + jt * 128
                    nc.tensor.matmul(
                        sps[:, c0:512],
                        lhsT=kT_sb[:, hh * NT + jtok:hh * NT + jtok + 128],
                        rhs=qT_sb[:, hh * NT + tok0 + c0:hh * NT + tok0 + 512],
                        start=True, stop=True,
                    )
                    es = ep.tile([128, 512], BF16, tag="es")
                    gt = b * 16 + jt
                    nc.scalar.activation(es[:, c0:512], sps[:, c0:512], AF.Exp,
                                         scale=rinvK_sb[:, hh * 32 + gt:
                                                        hh * 32 + gt + 1])
                    if s >= 0:
                        nc.gpsimd.affine_select(
                            out=es[:, c0:512], in_=es[:, c0:512],
                            compare_op=mybir.AluOpType.is_ge,
                            fill=0.0, base=c0 - 128 * s, channel_multiplier=-1,
                            pattern=[[1, 512 - c0]],
                        )
                    state[("es", hh, jt)] = (es, c0, gt)
                    if jt > jlo:
                        do_av(hh, jt - 1)
                return u

            def mk_tail(hh):
                def u():
                    do_av(hh, jhi - 1)
                    ups = state[hh]
                    u_sb = up_.tile([96, 512], F32, tag="usb", name=f"usb{hh}")
                    nc.vector.tensor_copy(u_sb[:], ups[0:96, :])
                    # reciprocal_approx_fast mis-reads non-zero partition
                    # bases (HW-verified) — stage Z on partition 0 first.
                    zrow = zp.tile([1, 512], F32, tag="zrow", name=f"zrow{hh}")
                    nc.vector.tensor_copy(zrow[:], ups[96:97, :])
                    zi32 = zp.tile([1, 512], F32, tag="zi32", name=f"zi32{hh}")
                    nc.vector.reciprocal_approx_fast(zi32[:], zrow[:])
                    zib = zp.tile([1, 512], BF16, tag="zib", name=f"zib{hh}")
                    nc.vector.tensor_copy(zib[:], zi32[:])
                    bz = bp.tile([96, 512], BF16, tag="bz", name=f"bz{hh}")
                    nc.gpsimd.partition_broadcast(bz[:], zib[:])
                    ou = op_.tile([96, 512], BF16, tag="ou", name=f"ou{hh}")
                    nc.vector.tensor_mul(ou[:], u_sb[:], bz[:])
                    ous_out.append(ou)
                return u

            for hh in range(HPC):
                units += [mk_j(hh, jt) for jt in range(jlo, jhi)]
                if do_tails:
                    units.append(mk_tail(hh))
            return units

        def proj_units(b, ib, ous, use_act=False, alt_pool=None, alt_from=1):
            tok0 = b * T + ib * 512
            units = []

            def mk_ct(ct):
                def u():
                    # psA's banks are idle in the epilogue (and in the
                    # last third of a stream, once rope(3) has consumed
                    # the last qk accumulator) — alternating the proj
                    # accumulators between psW and psA deepens the ring
                    # so the PE never waits on the PSUM drain
                    pool = alt_pool if (alt_pool is not None
                                        and ct >= alt_from
                                        and ct % 2 == 1) else psW
                    tag = "pA" if pool is psA else "pW"
                    ops = pool.tile([128, 512], F32, tag=tag,
                                    name=f"ops{ct}")
                    for hh in range(HPC):
                        nc.tensor.matmul(
                            ops[:],
                            lhsT=wo_sb[:, hh * C + ct * 128:hh * C + ct * 128 + 128],
                            rhs=ous[hh][:],
                            start=(hh == 0), stop=(hh == HPC - 1),
                        )
                    ob = obp.tile([128, 512], BF16, tag="ob")
                    if use_act and ct % 2 == 0:
                        # ACT is idle in the epilogue — alternate the PSUM
                        # drain copies so psW frees twice as fast
                        nc.scalar.activation(ob[:], ops[:], AF.Copy)
                    else:
                        nc.vector.tensor_copy(ob[:], ops[:])
                    nc.sync.dma_start(
                        out=outT[ct * 128:(ct + 1) * 128, tok0:tok0 + 512],
                        in_=ob[:])
                return u
            return [mk_ct(ct) for ct in range(KT)]

        def weave(prim, sec, lead=3):
            """Interleave unit emission ~proportionally; the first `lead`
            prim units run before any sec unit so cross-stream dependency
            chains (rinv -> qnorm -> S) get runway before attention hits
            the PE queue."""
            ia, ib_ = 0, 0
            la, lb = len(prim), len(sec)
            while ia < la or ib_ < lb:
                if ib_ >= lb or (ia < la and (ia - lead) * lb <= ib_ * la):
                    prim[ia]()
                    ia += 1
                else:
                    sec[ib_]()
                    ib_ += 1

        # stream n: dma(n+1) issued first, then weave([qk/rope(n) with
        # inline rinv + qnorm, v(n), proj(n-2)], attn(n-1)).
        def new_state():
            return {"xbg": [], "sqs": {}, "pas": {}, "rinvs": {}}

        states = {}
        ous = {}
        states[0] = new_state()
        # block-0 x tiles are spread over both non-sync DMA rings: each
        # ring moves only ~100 GB/s with these 1KB-line descriptors, so
        # serializing all four tiles on one ring starves the first qk
        # matmuls. wq keeps the sync ring to itself.
        dma_unit(0, states[0],
                 engs=[nc.gpsimd, nc.scalar, nc.gpsimd, nc.scalar])
        load_tables()
        load_wv_wo()
        last = NBLK - 1
        for n in range(NBLK):
            st = states[n]
            if n + 1 < NBLK:
                states[n + 1] = new_state()
                dma_unit(n + 1, states[n + 1])
            if n == 0:
                load_tables_chunk(512, 1024)
            elif n == 1:
                load_tables_chunk(1024, 2048)
            elif n == 2:
                load_tables_chunk(2048, NT)
            prim = a_units(n, st)
            if n >= 2:
                # use_act: alternate the psum drain copies between DVE and
                # ACT so a drain queued behind other DVE work can't
                # head-of-line-block the next proj matmul's bank claim
                prim += proj_units(*divmod(n - 2, IB_PER_B), ous.pop(n - 2),
                                   use_act=True, alt_pool=psA, alt_from=7)
            sec = []
            if n >= 1:
                ous[n - 1] = []
                sec = attn_units(*divmod(n - 1, IB_PER_B), ous[n - 1])
            weave(prim, sec, lead=3 if n == 1 else 1)
            states.pop(n - 1, None)

        # epilogue: attn(last) woven with proj(last-1), then proj(last)
        n = last
        ous[n] = []
        sec = attn_units(*divmod(n, IB_PER_B), ous[n])
        prim = proj_units(*divmod(n - 1, IB_PER_B), ous.pop(n - 1),
                          use_act=True, alt_pool=psA)
        weave(prim, sec, lead=0)
        for u in proj_units(*divmod(n, IB_PER_B), ous.pop(n),
                            use_act=True, alt_pool=psA):
            u()

        if KDEBUG:
            nc.sync.dma_start(out=qTd[:, :], in_=qT_sb[:])
            nc.sync.dma_start(out=kTd[:, :], in_=kT_sb[:])
            nc.sync.dma_start(out=vd[:, :], in_=v_sb[:])
            nc.sync.dma_start(out=rinvKd[:, :], in_=rinvK_sb[:])

    nc.compile()
    return nc


def _get_nc():
    if "nc" not in _CACHE:
        _CACHE["nc"] = _build()
    return _CACHE["nc"]


# ------------------------------------------------------------------ entrypoint

def _run(inputs, trace=False, **kw):
    nc = _get_nc()
    in_maps = _make_in_maps(**inputs)
    res = run_bass_kernel_spmd(nc, in_maps, core_ids=list(range(NCORES)),
                               trace=trace, **kw)
    acc = np.zeros((C, NT), np.float64)
    for r in res.results:
        acc += r["outT"].astype(np.float64)
    out = np.ascontiguousarray(acc.T.astype(np.float32)).reshape(B, T, C)
    return out, res


def kernel(**inputs) -> np.ndarray:
    out, _ = _run(inputs, trace=False)
    return out



# revision 15
# speedup vs baseline: 1.0786x; 1.0786x over previous
"""Causal self-attention (RoPE-3D + QK-RMSNorm) on 8 TRN2 NeuronCores.

Tensor-parallel over heads: 2 heads per core. Host shards W_qkv rows /
W_out columns, replicates x (pre-transposed, bf16), precomputes fused
RoPE cos/sin tables, and sums the 8 per-core partial projection outputs.

Per-core device kernel (Bass/Tile, no collectives), all-bf16 matmuls:
  Phase A (per 512-token block): QKV projection from a single bf16 x
    stream, QK-RMSNorm via ones-matmul partition reduction + Rsqrt +
    matmul partition-broadcast, RoPE via resident fused tables +
    stream_shuffle. q,k feature-major bf16 [96, tokens]; v token-major
    bf16 [tokens, 96(+1 ones col)].
  Phase B (per 512 q-token i-block): S^T = k_tile^T q_block (bf16),
    exp on ACT (no max subtraction needed: |S| <= sqrt(D)), causal mask
    via affine_select with diagonal-tile column trimming, AV + softmax
    denominator via [v|1] matmul, 1/Z on DVE reciprocal_approx_fast,
    out-projection into a [C, tokens] partial the host sums across
    cores.
"""

import math
import os
from contextlib import ExitStack

import numpy as np
import ml_dtypes

import concourse.bass as bass
import concourse.mybir as mybir
import concourse.tile as tile
from concourse import bacc
from concourse.bass_utils import run_bass_kernel_spmd

B, T, C = 2, 2048, 1536
H, D = 16, 96
NT = B * T                    # 4096 tokens
NCORES = 8
HPC = H // NCORES             # heads per core
ROPE_BASE = 10000.0

F32 = mybir.dt.float32
BF16 = mybir.dt.bfloat16

KT = C // 128                 # 12 contraction tiles over C
NBLK = NT // 512              # 8 token blocks
IB_PER_B = T // 512           # 4 q i-blocks per batch
VSTRIDE = 32 * 97             # v_sb per-head columns: 32 token-tiles x (96+1)

_CACHE = {}


# ----------------------------------------------------------------- host side

def _host_tables(coords, token_type, q_scale, k_scale):
    tt = (np.asarray(token_type).reshape(NT) > 0)
    half = 16
    inv_freq = ROPE_BASE ** (-np.arange(half, dtype=np.float64) / half)
    cf = np.empty((NT, D), np.float64)
    sf = np.empty((NT, D), np.float64)
    cflat = np.asarray(coords).reshape(NT, 3).astype(np.float64)
    for a in range(3):
        ang = cflat[:, a:a + 1] * inv_freq[None, :]
        c, s = np.cos(ang), np.sin(ang)
        cf[:, a * 32:a * 32 + 16] = c
        cf[:, a * 32 + 16:a * 32 + 32] = c
        sf[:, a * 32:a * 32 + 16] = -s
        sf[:, a * 32 + 16:a * 32 + 32] = s
    cf[~tt] = 1.0
    sf[~tt] = 0.0
    pi = (np.arange(D) // 32) * 32 + (np.arange(D) + 16) % 32
    c0 = 1.0 / math.sqrt(D)
    q_scale = np.asarray(q_scale, np.float64)
    k_scale = np.asarray(k_scale, np.float64)
    bf = ml_dtypes.bfloat16
    cosq = np.ascontiguousarray((cf * (q_scale[None, :] * c0)).T).astype(bf)
    sinq = np.ascontiguousarray((sf * (q_scale[pi][None, :] * c0)).T).astype(bf)
    cosk = np.ascontiguousarray((cf * k_scale[None, :]).T).astype(bf)
    sink = np.ascontiguousarray((sf * k_scale[pi][None, :]).T).astype(bf)
    return cosq, sinq, cosk, sink


def _make_in_maps(x, coords, token_type, W_qkv, W_out, q_scale, k_scale):
    x = np.asarray(x, np.float32)
    W_qkv = np.asarray(W_qkv, np.float32)
    W_out = np.asarray(W_out, np.float32)
    xbT = np.ascontiguousarray(x.reshape(NT, C).T).astype(ml_dtypes.bfloat16)
    cosq, sinq, cosk, sink = _host_tables(coords, token_type, q_scale, k_scale)
    in_maps = []
    for ci in range(NCORES):
        h0 = HPC * ci
        rows = np.concatenate([
            W_qkv[h0 * D:(h0 + HPC) * D],
            W_qkv[C + h0 * D:C + (h0 + HPC) * D],
        ], axis=0)                                        # [384, C] q,k rows
        wqkvT = np.ascontiguousarray(rows.T).astype(ml_dtypes.bfloat16)
        wvT = np.ascontiguousarray(
            W_qkv[2 * C + h0 * D:2 * C + (h0 + HPC) * D].T
        ).astype(ml_dtypes.bfloat16)                      # [C, 192] bf16
        woT = np.ascontiguousarray(
            W_out[:, h0 * D:(h0 + HPC) * D].T
        ).astype(ml_dtypes.bfloat16)                      # [192, C] bf16
        in_maps.append({
            "xbT": xbT, "wqkvT": wqkvT, "wvT": wvT, "woT": woT,
            "cosq": cosq, "sinq": sinq, "cosk": cosk, "sink": sink,
        })
    return in_maps


# --------------------------------------------------------------- bass builder

SWAP16 = [(i + 16) % 32 for i in range(32)]

_COMBINED_ACT_SET = "natural_log_exp_and_others"


def _pin_act_tables():
    """Make the ACT table-set picker choose the combined exp+ln set for
    every Exp/Ln activation (one ACT_TABLE_LOAD for the whole kernel
    instead of two switches per block). Only set *membership* is masked;
    dict order — and therefore act_func_set_id numbering — is unchanged."""
    import concourse.bacc as bacc_mod
    import concourse.hw_specs as hw_specs
    if getattr(bacc_mod, "_act_tables_pinned", False):
        return
    orig = hw_specs.get_activation_tables
    AF = mybir.ActivationFunctionType

    def patched(arch):
        tables = orig(arch)
        if _COMBINED_ACT_SET in tables:
            for name, fns in tables.items():
                if name != _COMBINED_ACT_SET:
                    fns.discard(AF.Exp)
                    fns.discard(AF.Ln)
        return tables

    bacc_mod.get_activation_tables = patched
    bacc_mod._act_tables_pinned = True


def _build():
    _pin_act_tables()
    nc = bacc.Bacc("TRN2", target_bir_lowering=False, debug=False)
    AF = mybir.ActivationFunctionType

    xbT = nc.declare_dram_parameter("xbT", [C, NT], BF16, isOutput=False)
    wqkvT = nc.declare_dram_parameter("wqkvT", [C, 2 * HPC * D], BF16, isOutput=False)
    wvT = nc.declare_dram_parameter("wvT", [C, HPC * D], BF16, isOutput=False)
    woT = nc.declare_dram_parameter("woT", [HPC * D, C], BF16, isOutput=False)
    cosq = nc.declare_dram_parameter("cosq", [D, NT], BF16, isOutput=False)
    sinq = nc.declare_dram_parameter("sinq", [D, NT], BF16, isOutput=False)
    cosk = nc.declare_dram_parameter("cosk", [D, NT], BF16, isOutput=False)
    sink = nc.declare_dram_parameter("sink", [D, NT], BF16, isOutput=False)
    outT = nc.declare_dram_parameter("outT", [C, NT], BF16, isOutput=True)
    KDEBUG = bool(os.environ.get("KDEBUG"))
    TRIM = os.environ.get("KTRIM", "1") == "1"
    if KDEBUG:
        qTd = nc.declare_dram_parameter("qTd", [96, HPC * NT], BF16, isOutput=True)
        kTd = nc.declare_dram_parameter("kTd", [96, HPC * NT], BF16, isOutput=True)
        vd = nc.declare_dram_parameter("vd", [128, HPC * VSTRIDE], BF16, isOutput=True)
        rinvKd = nc.declare_dram_parameter("rinvKd", [128, HPC * 32], F32, isOutput=True)

    with ExitStack() as ctx:
        tc = ctx.enter_context(tile.TileContext(nc))
        resid = ctx.enter_context(tc.tile_pool(name="resid", bufs=1))
        xbp = ctx.enter_context(tc.tile_pool(name="xbp", bufs=8))
        wk = ctx.enter_context(tc.tile_pool(name="wk", bufs=4))
        wk2 = ctx.enter_context(tc.tile_pool(name="wk2", bufs=4))
        rq = ctx.enter_context(tc.tile_pool(name="rq", bufs=3))
        rk = ctx.enter_context(tc.tile_pool(name="rk", bufs=2))
        zp = ctx.enter_context(tc.tile_pool(name="zp", bufs=3))
        ep = ctx.enter_context(tc.tile_pool(name="ep", bufs=4))
        op_ = ctx.enter_context(tc.tile_pool(name="op", bufs=5))
        up_ = ctx.enter_context(tc.tile_pool(name="up", bufs=3))
        obp = ctx.enter_context(tc.tile_pool(name="obp", bufs=2))
        bp = ctx.enter_context(tc.tile_pool(name="bp", bufs=3))
        psA = ctx.enter_context(tc.tile_pool(name="psA", bufs=2, space="PSUM"))
        psS = ctx.enter_context(tc.tile_pool(name="psS", bufs=2, space="PSUM"))
        psU = ctx.enter_context(tc.tile_pool(name="psU", bufs=1, space="PSUM"))
        psW = ctx.enter_context(tc.tile_pool(name="psW", bufs=3, space="PSUM"))
        dp = ctx.enter_context(tc.tile_pool(name="dp", bufs=4, space="DRAM"))

        # ---- residents. DMA descriptor issue is serialized per engine
        # queue (~0.8us each), so spread the startup loads across engine
        # queues and order them so the first qk matmul can start as soon
        # as wq chunk 0 + x0 tile 0 land (~3us) instead of after all
        # resident loads (~19us). wq is split into 4 chunks paired with
        # the 4 x tiles of block 0; x block 0 goes on the idle vector
        # queue so its issue overlaps the wq issues on sync.
        wq_sb = resid.tile([128, KT * 384], BF16, tag="wq")
        for a in range(4):
            nc.sync.dma_start(
                out=wq_sb[:, a * 3 * 384:(a + 1) * 3 * 384].rearrange(
                    "p (a f) -> p a f", a=3),
                in_=wqkvT[a * 384:(a + 1) * 384, :].rearrange(
                    "(a p) f -> p a f", p=128),
            )
        wv_sb = resid.tile([128, KT * 192], BF16, tag="wv")
        wo_sb = resid.tile([96, HPC * C], BF16, tag="wo")

        def load_wv_wo():
            # gpsimd queue: keeps the scalar queue free for the stream-0
            # Square/rinv ACT chain (DMA descriptor issue is ~0.7us each)
            nc.gpsimd.dma_start(
                out=wv_sb[:].rearrange("p (a f) -> p a f", a=KT),
                in_=wvT[:, :].rearrange("(a p) f -> p a f", p=128),
            )
            nc.gpsimd.dma_start(
                out=wo_sb[:].rearrange("p (h f) -> p h f", h=HPC),
                in_=woT[:, :].rearrange("(h p) f -> p h f", p=96),
            )
        qT_sb = resid.tile([96, HPC * NT], BF16, tag="qT")
        kT_sb = resid.tile([96, HPC * NT], BF16, tag="kT")
        v_sb = resid.tile([128, HPC * VSTRIDE], BF16, tag="v")
        rinvK_sb = resid.tile([128, HPC * 32], F32, tag="rinvK")
        onesb_sb = resid.tile([128, 1], BF16, tag="onesb")
        nc.gpsimd.memset(onesb_sb[:], 1.0)
        eps_sb = resid.tile([1, 1], F32, tag="eps")
        nc.gpsimd.memset(eps_sb[:], 1e-6)
        # ones column of each v token-tile
        nc.gpsimd.memset(
            v_sb[:].rearrange("p (h t f) -> p h t f", h=HPC, t=32)[:, :, :, 96:97],
            1.0,
        )

        # resident RoPE tables: block-0 columns first (rope(0) needs them
        # ~5us in), the remainder deferred into streams 0-2 so the bulk
        # doesn't steal HBM bandwidth from the critical wq/x0 transfers
        tab_sb = {}
        TABS = (("cq", cosq), ("sq", sinq), ("ck", cosk), ("sk", sink))

        def load_tables():
            for nm, par in TABS:
                t = resid.tile([96, NT], BF16, tag=f"tab{nm}")
                nc.gpsimd.dma_start(out=t[:, 0:512], in_=par[0:96, 0:512])
                tab_sb[nm] = t

        def load_tables_chunk(lo, hi):
            for nm, par in TABS:
                nc.scalar.dma_start(out=tab_sb[nm][:, lo:hi],
                                    in_=par[0:96, lo:hi])

        # ---------------------------------------------------------- units

        def dma_unit(n, st, engs=None):
            """Load 4 bf16 x tiles [128, 3*512] for token block n."""
            for g3 in range(KT // 3):
                tb = xbp.tile([128, 3 * 512], BF16, tag="xb", name=f"xb{g3}")
                eng = engs[g3] if engs else nc.sync
                eng.dma_start(
                    out=tb[:].rearrange("p (a f) -> p a f", a=3),
                    in_=xbT[g3 * 384:(g3 + 1) * 384,
                            n * 512:(n + 1) * 512].rearrange(
                                "(a p) f -> p a f", p=128))
                st["xbg"].append(tb)

        def mk_qk(n, st, g):
            def u():
                pa = psA.tile([96, 512], F32, tag="pA", name=f"pa{g}")
                for kt in range(KT):
                    nc.tensor.matmul(
                        pa[:],
                        lhsT=wq_sb[:, kt * 384 + g * 96:kt * 384 + g * 96 + 96],
                        rhs=st["xbg"][kt // 3][:, (kt % 3) * 512:
                                               (kt % 3 + 1) * 512],
                        start=(kt == 0), stop=(kt == KT - 1),
                    )
                sq = wk.tile([96, 512], BF16, tag="sq", name=f"sq{g}")
                nc.scalar.activation(sq[:], pa[:], AF.Square)
                st["pas"][g] = pa
                st["sqs"][g] = sq
            return u

        def mk_rope(n, st, g):
            def u():
                qk = "q" if g < HPC else "k"
                hh = g % HPC
                pa = st["pas"][g]
                ctab = tab_sb["cq" if qk == "q" else "ck"]
                stab = tab_sb["sq" if qk == "q" else "sk"]
                csl = ctab[:, n * 512:(n + 1) * 512]
                ssl = stab[:, n * 512:(n + 1) * 512]
                m1 = wk2.tile([96, 512], BF16, tag="m1")
                nc.vector.tensor_mul(m1[:], pa[:], csl)
                qshf = wk2.tile([96, 512], F32, tag="qshf")
                nc.vector.stream_shuffle(qshf[:], pa[:], mask=SWAP16)
                qsh = wk2.tile([96, 512], BF16, tag="qsh")
                nc.vector.tensor_mul(qsh[:], qshf[:], ssl)
                dest = (qT_sb if qk == "q" else kT_sb)
                nc.vector.tensor_add(
                    dest[:, hh * NT + n * 512:hh * NT + (n + 1) * 512],
                    m1[:], qsh[:])
            return u

        def mk_v(n, st, m):
            def u():
                pv = psW.tile([128, HPC * 96], F32, tag="pW", name=f"pv{m}")
                for kt in range(KT):
                    nc.tensor.matmul(
                        pv[:],
                        lhsT=st["xbg"][kt // 3][:, (kt % 3) * 512 + m * 128:
                                                (kt % 3) * 512 + (m + 1) * 128],
                        rhs=wv_sb[:, kt * 192:(kt + 1) * 192],
                        start=(kt == 0), stop=(kt == KT - 1),
                    )
                tt = n * 4 + m
                for hh in range(HPC):
                    nc.vector.tensor_copy(
                        v_sb[:, hh * VSTRIDE + tt * 97:hh * VSTRIDE + tt * 97 + 96],
                        pv[:, hh * 96:(hh + 1) * 96])
            return u

        def mk_rinv(n, st, gs):
            """ssq reduction + rinv (= exp(-0.5*ln(mean+eps)); Ln and Exp
            share one ACT table set so no table switches) for head group
            gs, emitted mid-stream right after its rope so nothing is left
            on the stream boundary. k-side rows are transposed to column
            layout via a small DRAM round trip."""
            def u():
                for g in gs:
                    ssq = psW.tile([1, 512], F32, tag="pW", name=f"ssq{g}")
                    nc.tensor.matmul(ssq[:], lhsT=onesb_sb[0:96, 0:1],
                                     rhs=st["sqs"][g][:], start=True, stop=True)
                    lt = rk.tile([1, 512], F32, tag="lt", name=f"lt{g}")
                    nc.scalar.activation(lt[:], ssq[:], AF.Ln,
                                         scale=1.0 / D, bias=eps_sb[:])
                    if g < HPC:
                        rinv = rq.tile([1, 512], BF16, tag="row",
                                       name=f"rinv{g}")
                    else:
                        rinv = rk.tile([1, 512], F32, tag="rowk",
                                       name=f"rinv{g}")
                    nc.scalar.activation(rinv[:], lt[:], AF.Exp, scale=-0.5)
                    st["rinvs"][g] = rinv
                    if g >= HPC:
                        hh = g - HPC
                        scr = dp.tile([512], F32, tag="scr", name=f"scr{hh}")
                        nc.sync.dma_start(out=scr[:], in_=rinv[:])
                        nc.sync.dma_start(
                            out=rinvK_sb[:, hh * 32 + n * 4:
                                         hh * 32 + (n + 1) * 4],
                            in_=scr[:].rearrange("(c j) -> j c", j=128),
                        )
            return u

        def a_units(n, st):
            units = []
            for g in range(2 * HPC):
                units.append(mk_qk(n, st, g))
                units.append(mk_rope(n, st, g))
                if g == HPC - 1:
                    units.append(mk_rinv(n, st, range(HPC)))
                elif g == 2 * HPC - 1:
                    units.append(mk_rinv(n, st, range(HPC, 2 * HPC)))
            # q-norm applied in the SAME stream (right after the k-side
            # rinv unit) so attention on block n never has to wait for
            # anything emitted in stream n+1.
            units += qnorm_units(n, st)
            units += [mk_v(n, st, m) for m in range(4)]
            return units

        def qnorm_units(n, st):
            """Apply q RMS-norm to the rope output of block n in place."""
            def u():
                for g in range(HPC):
                    bq = bp.tile([96, 512], BF16, tag="bq", name=f"bq{g}")
                    nc.gpsimd.partition_broadcast(bq[:], st["rinvs"][g][:])
                    dslice = qT_sb[:, g * NT + n * 512:g * NT + (n + 1) * 512]
                    nc.vector.tensor_mul(dslice, dslice, bq[:])
            return [u]

        def attn_units(b, ib, ous_out, state=None, jlo=0, jhi=None,
                       do_tails=True):
            """Attention units for q i-block ib of batch b, j-tiles
            [jlo, jhi). Tail units compute the fully normalized ou tiles
            (appended to ous_out). `state` carries the PSUM accumulators
            when the j-range is split across emission points."""
            tok0 = b * T + ib * 512
            njt = 4 * ib + 4
            if jhi is None:
                jhi = njt
            units = []
            if state is None:
                state = {}

            def do_av(hh, jt):
                """AV accumulate for a j-tile whose es is already queued —
                emitted one unit behind the S matmul so the PE never
                head-of-line blocks on the exp."""
                ups = state[hh]
                es, c0, gt = state.pop(("es", hh, jt))
                nc.tensor.matmul(
                    ups[:, c0:512],
                    lhsT=v_sb[:, hh * VSTRIDE + gt * 97:
                              hh * VSTRIDE + gt * 97 + 97],
                    rhs=es[:, c0:512],
                    start=(jt == 0), stop=(jt == njt - 1),
                )

            def mk_j(hh, jt):
                def u():
                    if jt == 0:
                        state[hh] = psU.tile([97, 512], F32, tag="pS",
                                             name=f"ups{hh}")
                    s = jt - 4 * ib
                    c0 = 128 * s if (s > 0 and TRIM) else 0
                    sps = psS.tile([128, 512], F32, tag="pA2")
                    jtok = b * T + jt * 128
                    nc.tensor.matmul(
                        sps[:, c0:512],
                        lhsT=kT_sb[:, hh * NT + jtok:hh * NT + jtok + 128],
                        rhs=qT_sb[:, hh * NT + tok0 + c0:hh * NT + tok0 + 512],
                        start=True, stop=True,
                    )
                    es = ep.tile([128, 512], BF16, tag="es")
                    gt = b * 16 + jt
                    nc.scalar.activation(es[:, c0:512], sps[:, c0:512], AF.Exp,
                                         scale=rinvK_sb[:, hh * 32 + gt:
                                                        hh * 32 + gt + 1])
                    if s >= 0:
                        nc.gpsimd.affine_select(
                            out=es[:, c0:512], in_=es[:, c0:512],
                            compare_op=mybir.AluOpType.is_ge,
                            fill=0.0, base=c0 - 128 * s, channel_multiplier=-1,
                            pattern=[[1, 512 - c0]],
                        )
                    state[("es", hh, jt)] = (es, c0, gt)
                    if jt > jlo:
                        do_av(hh, jt - 1)
                return u

            def mk_tail(hh):
                def u():
                    do_av(hh, jhi - 1)
                    ups = state[hh]
                    u_sb = up_.tile([96, 512], F32, tag="usb", name=f"usb{hh}")
                    nc.vector.tensor_copy(u_sb[:], ups[0:96, :])
                    # reciprocal_approx_fast mis-reads non-zero partition
                    # bases (HW-verified) — stage Z on partition 0 first.
                    zrow = zp.tile([1, 512], F32, tag="zrow", name=f"zrow{hh}")
                    nc.vector.tensor_copy(zrow[:], ups[96:97, :])
                    zi32 = zp.tile([1, 512], F32, tag="zi32", name=f"zi32{hh}")
                    nc.vector.reciprocal_approx_fast(zi32[:], zrow[:])
                    zib = zp.tile([1, 512], BF16, tag="zib", name=f"zib{hh}")
                    nc.vector.tensor_copy(zib[:], zi32[:])
                    bz = bp.tile([96, 512], BF16, tag="bz", name=f"bz{hh}")
                    nc.gpsimd.partition_broadcast(bz[:], zib[:])
                    ou = op_.tile([96, 512], BF16, tag="ou", name=f"ou{hh}")
                    nc.vector.tensor_mul(ou[:], u_sb[:], bz[:])
                    ous_out.append(ou)
                return u

            for hh in range(HPC):
                units += [mk_j(hh, jt) for jt in range(jlo, jhi)]
                if do_tails:
                    units.append(mk_tail(hh))
            return units

        def proj_units(b, ib, ous, use_act=False, alt_pool=None, alt_from=1):
            tok0 = b * T + ib * 512
            units = []

            def mk_ct(ct):
                def u():
                    # psA's banks are idle in the epilogue (and in the
                    # last third of a stream, once rope(3) has consumed
                    # the last qk accumulator) — alternating the proj
                    # accumulators between psW and psA deepens the ring
                    # so the PE never waits on the PSUM drain
                    pool = alt_pool if (alt_pool is not None
                                        and ct >= alt_from
                                        and ct % 2 == 1) else psW
                    tag = "pA" if pool is psA else "pW"
                    ops = pool.tile([128, 512], F32, tag=tag,
                                    name=f"ops{ct}")
                    for hh in range(HPC):
                        nc.tensor.matmul(
                            ops[:],
                            lhsT=wo_sb[:, hh * C + ct * 128:hh * C + ct * 128 + 128],
                            rhs=ous[hh][:],
                            start=(hh == 0), stop=(hh == HPC - 1),
                        )
                    ob = obp.tile([128, 512], BF16, tag="ob")
                    if use_act and ct % 2 == 0:
                        # ACT is idle in the epilogue — alternate the PSUM
                        # drain copies so psW frees twice as fast
                        nc.scalar.activation(ob[:], ops[:], AF.Copy)
                    else:
                        nc.vector.tensor_copy(ob[:], ops[:])
                    nc.sync.dma_start(
                        out=outT[ct * 128:(ct + 1) * 128, tok0:tok0 + 512],
                        in_=ob[:])
                return u
            return [mk_ct(ct) for ct in range(KT)]

        def weave(prim, sec, lead=3):
            """Interleave unit emission ~proportionally; the first `lead`
            prim units run before any sec unit so cross-stream dependency
            chains (rinv -> qnorm -> S) get runway before attention hits
            the PE queue."""
            ia, ib_ = 0, 0
            la, lb = len(prim), len(sec)
            while ia < la or ib_ < lb:
                if ib_ >= lb or (ia < la and (ia - lead) * lb <= ib_ * la):
                    prim[ia]()
                    ia += 1
                else:
                    sec[ib_]()
                    ib_ += 1

        # stream n: dma(n+1) issued first, then weave([qk/rope(n) with
        # inline rinv + qnorm, v(n), proj(n-2)], attn(n-1)).
        def new_state():
            return {"xbg": [], "sqs": {}, "pas": {}, "rinvs": {}}

        states = {}
        ous = {}
        states[0] = new_state()
        # block-0 x tiles are spread over both non-sync DMA rings: each
        # ring moves only ~100 GB/s with these 1KB-line descriptors, so
        # serializing all four tiles on one ring starves the first qk
        # matmuls. wq keeps the sync ring to itself.
        dma_unit(0, states[0],
                 engs=[nc.gpsimd, nc.scalar, nc.gpsimd, nc.scalar])
        load_tables()
        load_wv_wo()
        last = NBLK - 1
        for n in range(NBLK):
            st = states[n]
            if n + 1 < NBLK:
                states[n + 1] = new_state()
                dma_unit(n + 1, states[n + 1])
            if n == 0:
                load_tables_chunk(512, 1024)
            elif n == 1:
                load_tables_chunk(1024, 2048)
            elif n == 2:
                load_tables_chunk(2048, NT)
            prim = a_units(n, st)
            if n >= 2:
                # use_act: alternate the psum drain copies between DVE and
                # ACT so a drain queued behind other DVE work can't
                # head-of-line-block the next proj matmul's bank claim
                prim += proj_units(*divmod(n - 2, IB_PER_B), ous.pop(n - 2),
                                   use_act=True)
            sec = []
            if n >= 1:
                ous[n - 1] = []
                sec = attn_units(*divmod(n - 1, IB_PER_B), ous[n - 1])
            weave(prim, sec, lead=3 if n == 1 else 1)
            states.pop(n - 1, None)

        # epilogue: attn(last) woven with proj(last-1), then proj(last)
        n = last
        ous[n] = []
        sec = attn_units(*divmod(n, IB_PER_B), ous[n])
        prim = proj_units(*divmod(n - 1, IB_PER_B), ous.pop(n - 1),
                          use_act=True, alt_pool=psA)
        weave(prim, sec, lead=0)
        for u in proj_units(*divmod(n, IB_PER_B), ous.pop(n),
                            use_act=True, alt_pool=psA):
            u()

        if KDEBUG:
            nc.sync.dma_start(out=qTd[:, :], in_=qT_sb[:])
            nc.sync.dma_start(out=kTd[:, :], in_=kT_sb[:])
            nc.sync.dma_start(out=vd[:, :], in_=v_sb[:])
            nc.sync.dma_start(out=rinvKd[:, :], in_=rinvK_sb[:])

    nc.compile()
    return nc


def _get_nc():
    if "nc" not in _CACHE:
        _CACHE["nc"] = _build()
    return _CACHE["nc"]


# ------------------------------------------------------------------ entrypoint

def _run(inputs, trace=False, **kw):
    nc = _get_nc()
    in_maps = _make_in_maps(**inputs)
    res = run_bass_kernel_spmd(nc, in_maps, core_ids=list(range(NCORES)),
                               trace=trace, **kw)
    acc = np.zeros((C, NT), np.float64)
    for r in res.results:
        acc += r["outT"].astype(np.float64)
    out = np.ascontiguousarray(acc.T.astype(np.float32)).reshape(B, T, C)
    return out, res


def kernel(**inputs) -> np.ndarray:
    out, _ = _run(inputs, trace=False)
    return out

